# revision 11
# baseline (speedup 1.0000x reference)
"""Trainium2 Bass kernel for nn_Dilation2D (101x101 grayscale dilation with a
parabolic structuring element).

Math: out[r, c] = max_{i,j} padded[i + c, j + r] + h[i, j] with
h[i, j] = -(z_i^2 + z_j^2) / (4 s) separable into f(i) + g(j), so the 2D
max-plus convolution factors into two 1D sliding passes:

  stage 1:  t[p, r] = max_j rowpad[p, j + r] + w[j]     (slide along columns)
  stage 2:  out[r, c] = max_i tpad[i + c, r] + w[i]     (slide along rows)

with w[k] = -(k - 50)^2 / (4 s) and sentinel (-1e30) padding instead of -inf.

Sharding: output rows are split across the 8 cores (13 rows each, 104 >= 101).
Each core runs both stages restricted to its 13 output rows -- no cross-core
communication. Stage 1 keeps input rows on partitions (101 used): one
broadcast-add (tensor_tensor over a [101, 13, 101] sliding-window AP) plus a
free-dim max-reduce. The [101, 13] result is transposed on the tensor engine,
sentinel-padded to [13, 208], and replicated into a [104, 113] layout
(partition P = cc*13 + r holds tpad[r, cc*13 : cc*13+113]) so stage 2 is
again one broadcast-add + free-dim max-reduce across 104 partitions.

NEFF-level constraint honored throughout: every non-Drain instruction can
carry at most ONE semaphore wait. Hence: inputs are packed into a single DMA,
tpad has a single producing engine (DVE), PE/DVE absorb the input-DMA sem
early, SP drains (multi-wait capable) absorb queue sems before DMA groups,
and seven single-element DVE copies absorb the gather-queue sems so stage 2's
tensor_tensor needs only the last one.
"""

import numpy as np

K = 101          # image/kernel size
PAD = 50
S = 13           # output rows per core
NCORES = 8
W = S + K - 1    # 113: window columns each core needs
XCOLS = 204      # padded row length fed to each core (7*13 + 113 = 204)
TCOLS = 208      # stage-2 padded t row length (>= 7*13 + 12 + 101 + 1)
SENT = np.float32(-1.0e30)

# packed input layout: [128, W + K + K] = [128, 315]
#   cols [0, W)            rows 0..100 : x (per-core window slice)
#   cols [W, W+K)          rows 0..127 : w replicated
#   cols [W+K, W+2K)       rows 0..100 : identity
PCOLS = W + K + K

_CACHE = {}


def _build_nc():
    import bass_rust as _br
    import concourse.bass as bass
    import concourse.mybir as mybir
    import concourse.tile as tile

    f32 = mybir.dt.float32
    add = mybir.AluOpType.add
    amax = mybir.AluOpType.max
    nc = bass.Bass(target_bir_lowering=False)

    xin = nc.dram_tensor("xin", [128, PCOLS], f32, kind="ExternalInput")
    out = nc.dram_tensor("out", [NCORES * S, S], f32, kind="ExternalOutput")

    with tile.TileContext(nc) as tc:
        with (
            tc.tile_pool(name="pool", bufs=1) as pool,
            tc.tile_pool(name="psum", bufs=1, space="PSUM") as psum_pool,
        ):
            pk = pool.tile([128, PCOLS], f32)
            d_in = nc.sync.dma_start(pk[:, :], xin[:, :])

            xs_off = 0
            ws_off = W
            id_off = W + K

            tmp1 = pool.tile([K, S * K], f32)
            t1 = pool.tile([K, S], f32)

            # PE absorb: consume the input-DMA dep on the PE clock early so
            # the transpose below needs only the DVE wait.
            tp_ps = psum_pool.tile([S, K], f32)
            nc.tensor.matmul(
                tp_ps[0:1, 0:1],
                bass.AP(pk.tensor, id_off, [[PCOLS, 1], [1, 1]]),
                bass.AP(pk.tensor, id_off, [[PCOLS, 1], [1, 1]]),
            )

            # stage 1: tmp1[p, r, j] = pk[p, xs_off + r + j] + w[j]
            xs_win = bass.AP(pk.tensor, xs_off, [[PCOLS, K], [1, S], [1, K]])
            ws_b1 = bass.AP(pk.tensor, ws_off, [[PCOLS, K], [0, S], [1, K]])
            tmp1_w = bass.AP(tmp1.tensor, 0, [[S * K, K], [K, S], [1, K]])
            nc.vector.tensor_tensor(tmp1_w, xs_win, ws_b1, add)
            nc.vector.tensor_reduce(
                t1[:, :], tmp1_w, axis=mybir.AxisListType.X, op=amax
            )

            # transpose: tp_ps[r, p] = t1[p, r]
            idn = bass.AP(pk.tensor, id_off, [[PCOLS, K], [1, K]])
            tr_mm = nc.tensor.transpose(tp_ps[:, :], t1[:, :], idn)

            # tpad[r, 50 + p] = t1[p, r], sentinel elsewhere. Both writes on
            # DVE so the gather DMAs have a single producing proc to wait on.
            tpad = pool.tile([S, TCOLS], f32)
            nc.vector.memset(tpad[:, :], float(SENT))
            nc.vector.tensor_copy(tpad[0:S, PAD : PAD + K], tp_ps[:, :])

            # SP drain absorbing the input DMA's queue sem so the gather that
            # round-robins back onto queue 0 does not need a queue-order wait.
            spd1 = nc.sync.drain()
            _br.add_dep_helper(spd1.ins, d_in.ins, sync=True, reason="absorb in-q")

            # gather X[cc*13 + r, m] = tpad[r, cc*13 + m]; one DMA per cc
            # (an SBUF dest can only cross partitions in AP dim 0, so the
            # (cc, r) product cannot be a single DMA).
            X = pool.tile([NCORES * S, W], f32)
            gathers = []
            for cc in range(NCORES):
                g = nc.sync.dma_start(
                    X[cc * S : (cc + 1) * S, :],
                    tpad[0:S, cc * S : cc * S + W],
                )
                # order after spd1 so the queue-order sem for the wrap-around
                # queue is already observed on SP
                _br.add_dep_helper(g.ins, spd1.ins, sync=False, reason="order")
                if gathers:
                    _br.add_dep_helper(
                        g.ins, gathers[-1].ins, sync=False, reason="order"
                    )
                gathers.append(g)

            tmp2 = pool.tile([NCORES * S, S * K], f32)
            osb = pool.tile([NCORES * S, S], f32)
            scrap = pool.tile([1, NCORES], f32)

            # DVE absorbs: the 8 gathers land on 8 different DMA-queue sems.
            # Absorb 7 of them with single-element partition-0 copies (DVE ops
            # must start on partition 0/32/64/96, so they read the input tile
            # and take a manufactured sync dep on their gather).
            prev_ab = None
            for cc in range(NCORES - 1):
                ab = nc.vector.tensor_copy(
                    scrap[0:1, cc : cc + 1], pk[0:1, cc : cc + 1]
                )
                _br.add_dep_helper(
                    ab.ins, gathers[cc].ins, sync=True, reason="absorb gather sem"
                )
                if prev_ab is not None:
                    _br.add_dep_helper(ab.ins, prev_ab.ins, sync=False, reason="order")
                prev_ab = ab

            # stage 2: tmp2[P, c, i] = X[P, c + i] + w[i]
            X_win = bass.AP(X.tensor, 0, [[W, NCORES * S], [1, S], [1, K]])
            ws_b2 = bass.AP(pk.tensor, ws_off, [[PCOLS, NCORES * S], [0, S], [1, K]])
            tmp2_w = bass.AP(tmp2.tensor, 0, [[S * K, NCORES * S], [K, S], [1, K]])
            tt2 = nc.vector.tensor_tensor(tmp2_w, X_win, ws_b2, add)
            _br.add_dep_helper(tt2.ins, prev_ab.ins, sync=False, reason="order")
            red2 = nc.vector.tensor_reduce(
                osb[:, :], tmp2_w, axis=mybir.AxisListType.X, op=amax
            )

            # SP absorber ladder: one drain per outstanding proc tick, each
            # carrying exactly one sem wait (the ISA allows only one), chained
            # with order-only edges so wait elision applies down the ladder.
            # After the ladder, the output DMA and the framework exit drain
            # need at most one fresh wait each.
            ladder_events = [red2] + gathers + [tr_mm]
            prev_l = None
            for ev in ladder_events:
                ld = nc.sync.drain()
                _br.add_dep_helper(ld.ins, ev.ins, sync=True, reason="ladder")
                if prev_l is not None:
                    _br.add_dep_helper(ld.ins, prev_l.ins, sync=False, reason="order")
                prev_l = ld

            # output DMA on SWDGE (gpsimd): fresh queue + fresh engine clock
            # so it carries exactly one wait (the DVE data dep).
            d_out = nc.gpsimd.dma_start(out[:, :], osb[:, :])
            ld = nc.sync.drain()
            _br.add_dep_helper(ld.ins, d_out.ins, sync=True, reason="ladder")
            _br.add_dep_helper(ld.ins, prev_l.ins, sync=False, reason="order")

    return nc


def _prep_in_maps(input, scale):
    inp = np.asarray(input, dtype=np.float32)
    s = np.float32(np.asarray(scale).reshape(()))

    z = (np.arange(K, dtype=np.float32) - np.float32(PAD)).astype(np.float32)
    zsq = (z * z).astype(np.float32)
    wvec = (-zsq / (np.float32(4.0) * s)).astype(np.float32)

    rowpad = np.full((K, XCOLS), SENT, dtype=np.float32)
    rowpad[:, PAD : PAD + K] = inp

    in_maps = []
    for k in range(NCORES):
        packed = np.zeros((128, PCOLS), dtype=np.float32)
        packed[:K, :W] = rowpad[:, S * k : S * k + W]
        packed[:, W : W + K] = wvec[None, :]
        packed[:K, W + K : W + 2 * K] = np.eye(K, dtype=np.float32)
        in_maps.append({"xin": packed})
    return in_maps


def _unshard(results):
    out_full = np.empty((K, K), dtype=np.float32)
    for k, res in enumerate(results):
        o = np.asarray(res["out"]).reshape(NCORES, S, S)  # [cc, r_loc, c_in]
        block = o.transpose(1, 0, 2).reshape(S, NCORES * S)  # [r_loc, c]
        r0 = S * k
        nrows = min(S, K - r0)
        if nrows <= 0:
            continue
        out_full[r0 : r0 + nrows, :] = block[:nrows, :K]
    return out_full


def kernel(input, scale):
    from concourse.bass_utils import run_bass_kernel_spmd

    if "nc" not in _CACHE:
        _CACHE["nc"] = _build_nc()
    nc = _CACHE["nc"]

    in_maps = _prep_in_maps(input, scale)
    res = run_bass_kernel_spmd(nc, in_maps, core_ids=list(range(NCORES)))
    return _unshard(res.results)


# revision 14
# speedup vs baseline: 1.0433x; 1.0433x over previous
"""Trainium2 Bass kernel for nn_Dilation2D (101x101 grayscale dilation with a
parabolic structuring element).

Math: out[r, c] = max_{i,j} padded[i + c, j + r] + h[i, j] with
h[i, j] = -(z_i^2 + z_j^2) / (4 s) separable into f(i) + g(j), so the 2D
max-plus convolution factors into two 1D sliding passes:

  stage 1:  t[p, r] = max_j rowpad[p, j + r] + w[j]     (slide along columns)
  stage 2:  out[r, c] = max_i tpad[i + c, r] + w[i]     (slide along rows)

with w[k] = -(k - 50)^2 / (4 s) and sentinel (-1e30) padding instead of -inf.

Sharding: output rows are split across the 8 cores (13 rows each, 104 >= 101).
Each core runs both stages restricted to its 13 output rows -- no cross-core
communication. Stage 1 keeps input rows on partitions (101 used): one
broadcast-add (tensor_tensor over a [101, 13, 101] sliding-window AP) plus a
free-dim max-reduce. The [101, 13] result is transposed on the tensor engine,
sentinel-padded to [13, 208], and replicated into a [104, 113] layout
(partition P = cc*13 + r holds tpad[r, cc*13 : cc*13+113]) so stage 2 is
again one broadcast-add + free-dim max-reduce across 104 partitions.

Engine budget: the only sizable DMA is the input image (45.7 KB); the w row
is broadcast across partitions by the tensor engine, and the identity matrix
for the PE transpose is built on-chip by gpsimd (memset + affine_select).
The 8 replication gathers are split between the two HWDGE issuers (SP and
ACT) so the two descriptor generators run in parallel.

NEFF-level constraint honored throughout: every instruction can carry at
most ONE semaphore wait. Cross-engine handoffs are therefore staged through
single-wait absorber ops (DVE engine_nops, tiny ACT copies, tiny PE matmuls),
and a chain of single-wait SP drains before the Tile exit pre-observes every
proc so the framework's exit drain also needs at most one wait.
"""

import numpy as np

K = 101          # image/kernel size
PAD = 50
S = 13           # output rows per core
NCORES = 8
W = S + K - 1    # 113: window columns each core needs
XCOLS = 204      # padded row length fed to each core (7*13 + 113 = 204)
TCOLS = 208      # stage-2 padded t row length (>= 7*13 + 12 + 101 + 1)
SENT = np.float32(-1.0e30)

_CACHE = {}


def _build_nc():
    import bass_rust as _br
    import concourse.bass as bass
    import concourse.mybir as mybir
    import concourse.tile as tile

    f32 = mybir.dt.float32
    add = mybir.AluOpType.add
    amax = mybir.AluOpType.max
    nc = bass.Bass(target_bir_lowering=False)

    x_in = nc.dram_tensor("x", [K, W], f32, kind="ExternalInput")
    w_in = nc.dram_tensor("w", [1, K], f32, kind="ExternalInput")
    out = nc.dram_tensor("out", [NCORES * S, S], f32, kind="ExternalOutput")

    def dep(a, b, sync=True, reason="dep"):
        _br.add_dep_helper(a.ins, b.ins, sync=sync, reason=reason)

    with tile.TileContext(nc) as tc:
        with (
            tc.tile_pool(name="pool", bufs=1) as pool,
            tc.tile_pool(name="psum", bufs=1, space="PSUM") as psum_pool,
        ):
            xs = pool.tile([K, W], f32)
            wr = pool.tile([1, K], f32)
            d_x = nc.sync.dma_start(xs[:, :], x_in[:, :])
            d_w = nc.scalar.dma_start(wr[:, :], w_in[:, :])

            # gpsimd builds the transpose identity + the ones row for the
            # w broadcast -- all off the critical path, during the input DMA.
            ones1 = pool.tile([1, NCORES * S], f32)
            ones_k = pool.tile([K, K], f32)
            idn = pool.tile([K, K], f32)
            nc.gpsimd.memset(ones1[:, :], 1.0)
            nc.gpsimd.memset(ones_k[:, :], 1.0)
            g_idn = nc.gpsimd.affine_select(
                idn[:, :],
                ones_k[:, :],
                [[1, K]],
                mybir.AluOpType.is_equal,
                0.0,
                base=0,
                channel_multiplier=-1,
            )

            # PE: absorb the w DMA, then broadcast w across 104 partitions.
            scr_ps = psum_pool.tile([1, 1], f32)
            wps = psum_pool.tile([NCORES * S, K], f32)
            tp_ps = psum_pool.tile([S, K], f32)
            p_abs = nc.tensor.matmul(scr_ps[0:1, 0:1], wr[0:1, 0:1], wr[0:1, 0:1])
            p_wb = nc.tensor.matmul(wps[:, :], ones1[:, :], wr[:, :])

            tmp1 = pool.tile([K, S * K], f32)
            t1 = pool.tile([K, S], f32)
            tpad = pool.tile([S, TCOLS], f32)

            # DVE: memset first (no deps), absorb the PE w-broadcast, then
            # stage 1 carries only the input-DMA wait.
            nc.vector.memset(tpad[:, :], float(SENT))
            vn_w = nc.vector.engine_nop()
            dep(vn_w, p_wb, reason="absorb w-bcast")

            # stage 1: tmp1[p, r, j] = xs[p, r + j] + w[j]
            xs_win = bass.AP(xs.tensor, 0, [[W, K], [1, S], [1, K]])
            ws_b1 = bass.AP(wps.tensor, 0, [[K, K], [0, S], [1, K]])
            tmp1_w = bass.AP(tmp1.tensor, 0, [[S * K, K], [K, S], [1, K]])
            nc.vector.tensor_tensor(tmp1_w, xs_win, ws_b1, add)
            red1 = nc.vector.tensor_reduce(
                t1[:, :], tmp1_w, axis=mybir.AxisListType.X, op=amax
            )

            # PE: absorb the gpsimd identity, then transpose t1 -> tp_ps.
            p_abs2 = nc.tensor.matmul(scr_ps[0:1, 0:1], idn[0:1, 0:1], idn[0:1, 0:1])
            dep(p_abs2, g_idn, sync=False, reason="order")
            tr_mm = nc.tensor.transpose(tp_ps[:, :], t1[:, :], idn[:, :])

            # tpad[r, 50 + p] = t1[p, r]
            v_cp = nc.vector.tensor_copy(tpad[0:S, PAD : PAD + K], tp_ps[:, :])

            # gather X[cc*13 + r, m] = tpad[r, cc*13 + m]; one DMA per cc
            # (an SBUF dest can only cross partitions in AP dim 0, so the
            # (cc, r) product cannot be a single DMA). Split across the two
            # HWDGE issuers so the descriptor generators run in parallel.
            X = pool.tile([NCORES * S, W], f32)
            scrA = pool.tile([1, 8], f32)

            # ACT absorbs so ACT-issued gathers never need a fresh queue-order
            # wait on a queue first used by d_x / d_w.
            a_abs1 = nc.scalar.copy(scrA[0:1, 0:1], xs[0:1, 0:1])
            a_abs2 = nc.scalar.copy(scrA[0:1, 1:2], wr[0:1, 0:1])

            gathers = []
            for cc in range(NCORES):
                eng = nc.sync if cc < 4 else nc.scalar
                g = eng.dma_start(
                    X[cc * S : (cc + 1) * S, :],
                    tpad[0:S, cc * S : cc * S + W],
                )
                gathers.append(g)

            tmp2 = pool.tile([NCORES * S, S * K], f32)
            osb = pool.tile([NCORES * S, S], f32)

            # DVE absorbs: the 8 gathers land on 8 different DMA-queue sems;
            # absorb 7 with engine_nops so stage 2's tensor_tensor needs only
            # the last one.
            prev = None
            for cc in range(NCORES - 1):
                ab = nc.vector.engine_nop()
                dep(ab, gathers[cc], reason="absorb gather sem")
                if prev is not None:
                    dep(ab, prev, sync=False, reason="order")
                prev = ab

            # stage 2: tmp2[P, c, i] = X[P, c + i] + w[i]
            X_win = bass.AP(X.tensor, 0, [[W, NCORES * S], [1, S], [1, K]])
            ws_b2 = bass.AP(wps.tensor, 0, [[K, NCORES * S], [0, S], [1, K]])
            tmp2_w = bass.AP(tmp2.tensor, 0, [[S * K, NCORES * S], [K, S], [1, K]])
            tt2 = nc.vector.tensor_tensor(tmp2_w, X_win, ws_b2, add)
            dep(tt2, prev, sync=False, reason="order")
            red2 = nc.vector.tensor_reduce(
                osb[:, :], tmp2_w, axis=mybir.AxisListType.X, op=amax
            )

            # ACT absorbs the final DVE tick and the queue its out-DMA will
            # reuse, then issues the output DMA with no fresh waits.
            a_abs3 = nc.scalar.copy(scrA[0:1, 2:3], osb[0:1, 0:1])
            dep(a_abs3, red2, reason="absorb red2")
            d_out = nc.scalar.dma_start(out[:, :], osb[:, :])
            dep(d_out, a_abs3, sync=False, reason="order")

            # SP drain ladder: pre-observe every proc's final tick, one
            # single-wait drain at a time, so the framework exit drain needs
            # at most one fresh wait.
            ladder_events = (
                [d_x, d_w, g_idn, p_wb, tr_mm, red2]
                + gathers
                + [a_abs3, d_out]
            )
            prev_l = None
            for ev in ladder_events:
                ld = nc.sync.drain()
                dep(ld, ev, reason="ladder")
                if prev_l is not None:
                    dep(ld, prev_l, sync=False, reason="order")
                prev_l = ld

    return nc


def _prep_in_maps(input, scale):
    inp = np.asarray(input, dtype=np.float32)
    s = np.float32(np.asarray(scale).reshape(()))

    z = (np.arange(K, dtype=np.float32) - np.float32(PAD)).astype(np.float32)
    zsq = (z * z).astype(np.float32)
    wvec = (-zsq / (np.float32(4.0) * s)).astype(np.float32)

    rowpad = np.full((K, XCOLS), SENT, dtype=np.float32)
    rowpad[:, PAD : PAD + K] = inp

    in_maps = []
    for k in range(NCORES):
        in_maps.append(
            {
                "x": np.ascontiguousarray(rowpad[:, S * k : S * k + W]),
                "w": wvec[None, :].copy(),
            }
        )
    return in_maps


def _unshard(results):
    out_full = np.empty((K, K), dtype=np.float32)
    for k, res in enumerate(results):
        o = np.asarray(res["out"]).reshape(NCORES, S, S)  # [cc, r_loc, c_in]
        block = o.transpose(1, 0, 2).reshape(S, NCORES * S)  # [r_loc, c]
        r0 = S * k
        nrows = min(S, K - r0)
        if nrows <= 0:
            continue
        out_full[r0 : r0 + nrows, :] = block[:nrows, :K]
    return out_full


def kernel(input, scale):
    from concourse.bass_utils import run_bass_kernel_spmd

    if "nc" not in _CACHE:
        _CACHE["nc"] = _build_nc()
    nc = _CACHE["nc"]

    in_maps = _prep_in_maps(input, scale)
    res = run_bass_kernel_spmd(nc, in_maps, core_ids=list(range(NCORES)))
    return _unshard(res.results)


# revision 16
# speedup vs baseline: 1.0865x; 1.0414x over previous
"""Trainium2 Bass kernel for nn_Dilation2D (101x101 grayscale dilation with a
parabolic structuring element).

Math: out[r, c] = max_{i,j} padded[i + c, j + r] + h[i, j] with
h[i, j] = -(z_i^2 + z_j^2) / (4 s) separable into f(i) + g(j), so the 2D
max-plus convolution factors into two 1D sliding passes:

  stage 1:  t[p, r] = max_j rowpad[p, j + r] + w[j]     (slide along columns)
  stage 2:  out[r, c] = max_i tpad[i + c, r] + w[i]     (slide along rows)

with w[k] = -(k - 50)^2 / (4 s) and sentinel (-1e30) padding instead of -inf.

Sharding: output rows are split across the 8 cores (13 rows each, 104 >= 101).
Each core runs both stages restricted to its 13 output rows -- no cross-core
communication. Stage 1 keeps input rows on partitions (101 used): one
broadcast-add (tensor_tensor over a [101, 13, 101] sliding-window AP) plus a
free-dim max-reduce. The [101, 13] result is transposed on the tensor engine,
sentinel-padded to [13, 208], and replicated into a [104, 113] layout
(partition P = cc*13 + r holds tpad[r, cc*13 : cc*13+113]) so stage 2 is
again one broadcast-add + free-dim max-reduce across 104 partitions.

Implementation is raw Bass (no Tile framework): manual semaphores avoid the
Tile entry/exit barrier overhead (~12 us on this toolchain), and all eight
replication gathers increment one shared semaphore so the single-sem-wait
ISA limit is satisfied with standalone wait instructions. The w row is
broadcast across partitions by the tensor engine and the transpose identity
is built on-chip by gpsimd, so the only sizable DMA is the input image
(45.7 KB, split over two queues). Gathers are split between the two HWDGE
issuers (SP and ACT) so the descriptor generators run in parallel.
"""

import numpy as np

K = 101          # image/kernel size
PAD = 50
S = 13           # output rows per core
NCORES = 8
W = S + K - 1    # 113: window columns each core needs
XCOLS = 204      # padded row length fed to each core (7*13 + 113 = 204)
TCOLS = 208      # stage-2 padded t row length (>= 7*13 + 12 + 101 + 1)
SENT = np.float32(-1.0e30)

_CACHE = {}


def _build_nc():
    import concourse.bass as bass
    import concourse.mybir as mybir

    f32 = mybir.dt.float32
    add = mybir.AluOpType.add
    amax = mybir.AluOpType.max
    nc = bass.Bass(target_bir_lowering=False)

    x_in = nc.dram_tensor("x", [K, W], f32, kind="ExternalInput")
    w_in = nc.dram_tensor("w", [1, K], f32, kind="ExternalInput")
    out = nc.dram_tensor("out", [NCORES * S, S], f32, kind="ExternalOutput")

    with (
        nc.sbuf_tensor("xs", [K, W], f32) as xs,
        nc.sbuf_tensor("wr", [1, K], f32) as wr,
        nc.sbuf_tensor("ones1", [1, NCORES * S], f32) as ones1,
        nc.sbuf_tensor("ones_k", [K, K], f32) as ones_k,
        nc.sbuf_tensor("idn", [K, K], f32) as idn,
        nc.sbuf_tensor("tmp1", [K, S * K], f32) as tmp1,
        nc.sbuf_tensor("t1", [K, S], f32) as t1,
        nc.sbuf_tensor("tpad", [S, TCOLS], f32) as tpad,
        nc.sbuf_tensor("X", [NCORES * S, W], f32) as X,
        nc.sbuf_tensor("tmp2", [NCORES * S, S * K], f32) as tmp2,
        nc.sbuf_tensor("osb", [NCORES * S, S], f32) as osb,
        nc.psum_tensor("wps", [NCORES * S, K], f32) as wps,
        nc.psum_tensor("tp_ps", [S, K], f32) as tp_ps,
        nc.semaphore("s_dx") as s_dx,
        nc.semaphore("s_dw") as s_dw,
        nc.semaphore("s_idn") as s_idn,
        nc.semaphore("s_pe") as s_pe,
        nc.semaphore("s_dve") as s_dve,
        nc.semaphore("s_g") as s_g,
        nc.semaphore("s_out") as s_out,
        nc.Block() as block,
    ):
        xs_win = bass.AP(xs, 0, [[W, K], [1, S], [1, K]])
        ws_b1 = bass.AP(wps, 0, [[K, K], [0, S], [1, K]])
        tmp1_w = bass.AP(tmp1, 0, [[S * K, K], [K, S], [1, K]])
        X_win = bass.AP(X, 0, [[W, NCORES * S], [1, S], [1, K]])
        ws_b2 = bass.AP(wps, 0, [[K, NCORES * S], [0, S], [1, K]])
        tmp2_w = bass.AP(tmp2, 0, [[S * K, NCORES * S], [K, S], [1, K]])

        @block.sync
        def _(sync):
            # split the input image across two queues for transfer overlap
            sync.dma_start(
                bass.AP(xs, 0, [[W, 51], [1, W]]),
                bass.AP(x_in, 0, [[W, 51], [1, W]]),
            ).then_inc(s_dx, 16)
            sync.dma_start(
                bass.AP(xs, 51 * W, [[W, 50], [1, W]]),
                bass.AP(x_in, 51 * W, [[W, 50], [1, W]]),
            ).then_inc(s_dx, 16)
            # gathers cc = 0..3
            sync.wait_ge(s_dve, 2)
            for cc in range(4):
                sync.dma_start(
                    X[cc * S : (cc + 1) * S, :],
                    tpad[0:S, cc * S : cc * S + W],
                ).then_inc(s_g, 16)

        @block.scalar
        def _(scalar):
            scalar.dma_start(wr[:, :], w_in[:, :]).then_inc(s_dw, 16)
            # gathers cc = 4..7
            scalar.wait_ge(s_dve, 2)
            for cc in range(4, NCORES):
                scalar.dma_start(
                    X[cc * S : (cc + 1) * S, :],
                    tpad[0:S, cc * S : cc * S + W],
                ).then_inc(s_g, 16)
            scalar.wait_ge(s_dve, 3)
            scalar.dma_start(out[:, :], osb[:, :]).then_inc(s_out, 16)
            scalar.wait_ge(s_out, 16)

        @block.gpsimd
        def _(gpsimd):
            gpsimd.memset(ones1[:, :], 1.0)
            gpsimd.memset(ones_k[:, :], 1.0)
            gpsimd.drain()
            gpsimd.affine_select(
                idn[:, :],
                ones_k[:, :],
                [[1, K]],
                mybir.AluOpType.is_equal,
                0.0,
                base=0,
                channel_multiplier=-1,
            ).then_inc(s_idn, 1)

        @block.tensor
        def _(tensor):
            tensor.wait_ge(s_idn, 1)
            tensor.wait_ge(s_dw, 16)
            tensor.matmul(wps[:, :], ones1[:, :], wr[:, :]).then_inc(s_pe, 1)
            tensor.wait_ge(s_dve, 1)
            tensor.transpose(tp_ps[:, :], t1[:, :], idn[:, :]).then_inc(s_pe, 1)

        @block.vector
        def _(vector):
            vector.memset(tpad[:, :], float(SENT))
            vector.wait_ge(s_pe, 1)
            vector.wait_ge(s_dx, 32)
            # stage 1: tmp1[p, r, j] = xs[p, r + j] + w[j]
            vector.tensor_tensor(tmp1_w, xs_win, ws_b1, add)
            vector.drain()
            vector.tensor_reduce(
                t1[:, :], tmp1_w, axis=mybir.AxisListType.X, op=amax
            ).then_inc(s_dve, 1)
            vector.wait_ge(s_pe, 2)
            vector.drain()
            # tpad[r, 50 + p] = t1[p, r]
            vector.tensor_copy(tpad[0:S, PAD : PAD + K], tp_ps[:, :]).then_inc(
                s_dve, 1
            )
            vector.wait_ge(s_g, 128)
            # stage 2: tmp2[P, c, i] = X[P, c + i] + w[i]
            vector.tensor_tensor(tmp2_w, X_win, ws_b2, add)
            vector.drain()
            vector.tensor_reduce(
                osb[:, :], tmp2_w, axis=mybir.AxisListType.X, op=amax
            ).then_inc(s_dve, 1)

    return nc


def _prep_in_maps(input, scale):
    inp = np.asarray(input, dtype=np.float32)
    s = np.float32(np.asarray(scale).reshape(()))

    z = (np.arange(K, dtype=np.float32) - np.float32(PAD)).astype(np.float32)
    zsq = (z * z).astype(np.float32)
    wvec = (-zsq / (np.float32(4.0) * s)).astype(np.float32)

    rowpad = np.full((K, XCOLS), SENT, dtype=np.float32)
    rowpad[:, PAD : PAD + K] = inp

    in_maps = []
    for k in range(NCORES):
        in_maps.append(
            {
                "x": np.ascontiguousarray(rowpad[:, S * k : S * k + W]),
                "w": wvec[None, :].copy(),
            }
        )
    return in_maps


def _unshard(results):
    out_full = np.empty((K, K), dtype=np.float32)
    for k, res in enumerate(results):
        o = np.asarray(res["out"]).reshape(NCORES, S, S)  # [cc, r_loc, c_in]
        block = o.transpose(1, 0, 2).reshape(S, NCORES * S)  # [r_loc, c]
        r0 = S * k
        nrows = min(S, K - r0)
        if nrows <= 0:
            continue
        out_full[r0 : r0 + nrows, :] = block[:nrows, :K]
    return out_full


def kernel(input, scale):
    from concourse.bass_utils import run_bass_kernel_spmd

    if "nc" not in _CACHE:
        _CACHE["nc"] = _build_nc()
    nc = _CACHE["nc"]

    in_maps = _prep_in_maps(input, scale)
    res = run_bass_kernel_spmd(nc, in_maps, core_ids=list(range(NCORES)))
    return _unshard(res.results)


# revision 17
# speedup vs baseline: 1.1184x; 1.0294x over previous
"""Trainium2 Bass kernel for nn_Dilation2D (101x101 grayscale dilation with a
parabolic structuring element).

Math: out[r, c] = max_{i,j} padded[i + c, j + r] + h[i, j] with
h[i, j] = -(z_i^2 + z_j^2) / (4 s) separable into f(i) + g(j), so the 2D
max-plus convolution factors into two 1D sliding passes:

  stage 1:  t[p, r] = max_j rowpad[p, j + r] + w[j]     (slide along columns)
  stage 2:  out[r, c] = max_i tpad[i + c, r] + w[i]     (slide along rows)

with w[k] = -(k - 50)^2 / (4 s) and sentinel (-1e30) padding instead of -inf.

Sharding: output rows are split across the 8 cores (13 rows each, 104 >= 101).
Each core runs both stages restricted to its 13 output rows -- no cross-core
communication. Stage 1 keeps input rows on partitions (101 used): one
broadcast-add (tensor_tensor over a [101, 13, 101] sliding-window AP) plus a
free-dim max-reduce. The [101, 13] result is transposed on the tensor engine,
sentinel-padded to [13, 224], and replicated into a [104, 128] layout
(partition P = cc*13 + r holds tpad[r, cc*13 : cc*13+128]) so stage 2 is
again one broadcast-add + free-dim max-reduce across 104 partitions.

Implementation is raw Bass (no Tile framework): manual semaphores avoid the
Tile entry/exit barrier overhead (~12 us on this toolchain), and all eight
replication gathers increment one shared semaphore so the single-sem-wait
ISA limit is satisfied with standalone wait instructions. The w row is
broadcast across partitions by the tensor engine and the transpose identity
is built on-chip by gpsimd, so the only sizable DMA is the input image.
Every DMA row is exactly 512 B (128 f32) -- below that the DMA pays a 2x
latency multiplier. Gathers are split between the two HWDGE issuers (SP and
ACT) so the descriptor generators run in parallel.
"""

import numpy as np

K = 101          # image/kernel size
PAD = 50
S = 13           # output rows per core
NCORES = 8
W = S + K - 1    # 113: window columns each core needs for compute
WT = 128         # transfer width: 512-byte rows for full DMA bandwidth
XCOLS = 224      # host-side padded row length (>= 7*13 + 128)
TCOLS = 224      # stage-2 padded t row length (>= 7*13 + 128)
SENT = np.float32(-1.0e30)

_CACHE = {}


def _build_nc():
    import concourse.bass as bass
    import concourse.mybir as mybir

    f32 = mybir.dt.float32
    add = mybir.AluOpType.add
    amax = mybir.AluOpType.max
    nc = bass.Bass(target_bir_lowering=False, debug=False, enable_asserts=False)

    x_in = nc.dram_tensor("x", [K, WT], f32, kind="ExternalInput")
    w_in = nc.dram_tensor("w", [1, K], f32, kind="ExternalInput")
    out = nc.dram_tensor("out", [NCORES * S, S], f32, kind="ExternalOutput")

    with (
        nc.sbuf_tensor("xs", [K, WT], f32) as xs,
        nc.sbuf_tensor("wr", [1, K], f32) as wr,
        nc.sbuf_tensor("ones1", [1, NCORES * S], f32) as ones1,
        nc.sbuf_tensor("ones_k", [K, K], f32) as ones_k,
        nc.sbuf_tensor("idn", [K, K], f32) as idn,
        nc.sbuf_tensor("tmp1", [K, S * K], f32) as tmp1,
        nc.sbuf_tensor("t1", [K, S], f32) as t1,
        nc.sbuf_tensor("tpad", [S, TCOLS], f32) as tpad,
        nc.sbuf_tensor("X", [NCORES * S, WT], f32) as X,
        nc.sbuf_tensor("tmp2", [NCORES * S, S * K], f32) as tmp2,
        nc.sbuf_tensor("osb", [NCORES * S, S], f32) as osb,
        nc.psum_tensor("wps", [NCORES * S, K], f32) as wps,
        nc.psum_tensor("tp_ps", [S, K], f32) as tp_ps,
        nc.semaphore("s_dx") as s_dx,
        nc.semaphore("s_dw") as s_dw,
        nc.semaphore("s_idn") as s_idn,
        nc.semaphore("s_pe") as s_pe,
        nc.semaphore("s_dve") as s_dve,
        nc.semaphore("s_g") as s_g,
        nc.semaphore("s_out") as s_out,
        nc.Block() as block,
    ):
        xs_win = bass.AP(xs, 0, [[WT, K], [1, S], [1, K]])
        ws_b1 = bass.AP(wps, 0, [[K, K], [0, S], [1, K]])
        tmp1_w = bass.AP(tmp1, 0, [[S * K, K], [K, S], [1, K]])
        X_win = bass.AP(X, 0, [[WT, NCORES * S], [1, S], [1, K]])
        ws_b2 = bass.AP(wps, 0, [[K, NCORES * S], [0, S], [1, K]])
        tmp2_w = bass.AP(tmp2, 0, [[S * K, NCORES * S], [K, S], [1, K]])

        @block.sync
        def _(sync):
            # first half of the input image (rows 0..50)
            sync.dma_start(
                bass.AP(xs, 0, [[WT, 51], [1, WT]]),
                bass.AP(x_in, 0, [[WT, 51], [1, WT]]),
            ).then_inc(s_dx, 16)
            # gathers cc = 0..3
            sync.wait_ge(s_dve, 2)
            for cc in range(4):
                sync.dma_start(
                    X[cc * S : (cc + 1) * S, :],
                    tpad[0:S, cc * S : cc * S + WT],
                ).then_inc(s_g, 16)

        @block.scalar
        def _(scalar):
            scalar.dma_start(wr[:, :], w_in[:, :]).then_inc(s_dw, 16)
            # second half of the input image (rows 51..100)
            scalar.dma_start(
                bass.AP(xs, 51 * WT, [[WT, 50], [1, WT]]),
                bass.AP(x_in, 51 * WT, [[WT, 50], [1, WT]]),
            ).then_inc(s_dx, 16)
            # gathers cc = 4..7
            scalar.wait_ge(s_dve, 2)
            for cc in range(4, NCORES):
                scalar.dma_start(
                    X[cc * S : (cc + 1) * S, :],
                    tpad[0:S, cc * S : cc * S + WT],
                ).then_inc(s_g, 16)
            scalar.wait_ge(s_dve, 3)
            scalar.dma_start(out[:, :], osb[:, :]).then_inc(s_out, 16)

        @block.gpsimd
        def _(gpsimd):
            gpsimd.memset(ones1[:, :], 1.0)
            gpsimd.memset(ones_k[:, :], 1.0)
            gpsimd.drain()
            gpsimd.affine_select(
                idn[:, :],
                ones_k[:, :],
                [[1, K]],
                mybir.AluOpType.is_equal,
                0.0,
                base=0,
                channel_multiplier=-1,
            ).then_inc(s_idn, 1)

        @block.tensor
        def _(tensor):
            tensor.wait_ge(s_idn, 1)
            tensor.wait_ge(s_dw, 16)
            tensor.matmul(wps[:, :], ones1[:, :], wr[:, :]).then_inc(s_pe, 1)
            tensor.wait_ge(s_dve, 1)
            tensor.transpose(tp_ps[:, :], t1[:, :], idn[:, :]).then_inc(s_pe, 1)

        @block.vector
        def _(vector):
            vector.memset(tpad[:, :], float(SENT))
            vector.wait_ge(s_pe, 1)
            vector.wait_ge(s_dx, 32)
            # stage 1: tmp1[p, r, j] = xs[p, r + j] + w[j]
            vector.tensor_tensor(tmp1_w, xs_win, ws_b1, add)
            vector.drain()
            vector.tensor_reduce(
                t1[:, :], tmp1_w, axis=mybir.AxisListType.X, op=amax
            ).then_inc(s_dve, 1)
            vector.wait_ge(s_pe, 2)
            vector.drain()
            # tpad[r, 50 + p] = t1[p, r]
            vector.tensor_copy(tpad[0:S, PAD : PAD + K], tp_ps[:, :]).then_inc(
                s_dve, 1
            )
            vector.wait_ge(s_g, 128)
            # stage 2: tmp2[P, c, i] = X[P, c + i] + w[i]
            vector.tensor_tensor(tmp2_w, X_win, ws_b2, add)
            vector.drain()
            vector.tensor_reduce(
                osb[:, :], tmp2_w, axis=mybir.AxisListType.X, op=amax
            ).then_inc(s_dve, 1)

    return nc


def _prep_in_maps(input, scale):
    inp = np.asarray(input, dtype=np.float32)
    s = np.float32(np.asarray(scale).reshape(()))

    z = (np.arange(K, dtype=np.float32) - np.float32(PAD)).astype(np.float32)
    zsq = (z * z).astype(np.float32)
    wvec = (-zsq / (np.float32(4.0) * s)).astype(np.float32)

    rowpad = np.full((K, XCOLS), SENT, dtype=np.float32)
    rowpad[:, PAD : PAD + K] = inp

    in_maps = []
    for k in range(NCORES):
        in_maps.append(
            {
                "x": np.ascontiguousarray(rowpad[:, S * k : S * k + WT]),
                "w": wvec[None, :].copy(),
            }
        )
    return in_maps


def _unshard(results):
    out_full = np.empty((K, K), dtype=np.float32)
    for k, res in enumerate(results):
        o = np.asarray(res["out"]).reshape(NCORES, S, S)  # [cc, r_loc, c_in]
        block = o.transpose(1, 0, 2).reshape(S, NCORES * S)  # [r_loc, c]
        r0 = S * k
        nrows = min(S, K - r0)
        if nrows <= 0:
            continue
        out_full[r0 : r0 + nrows, :] = block[:nrows, :K]
    return out_full


def kernel(input, scale):
    from concourse.bass_utils import run_bass_kernel_spmd

    if "nc" not in _CACHE:
        _CACHE["nc"] = _build_nc()
    nc = _CACHE["nc"]

    in_maps = _prep_in_maps(input, scale)
    res = run_bass_kernel_spmd(nc, in_maps, core_ids=list(range(NCORES)))
    return _unshard(res.results)


# revision 18
# speedup vs baseline: 1.1631x; 1.0399x over previous
"""Trainium2 Bass kernel for nn_Dilation2D (101x101 grayscale dilation with a
parabolic structuring element).

Math: out[r, c] = max_{i,j} padded[i + c, j + r] + h[i, j] with
h[i, j] = -(z_i^2 + z_j^2) / (4 s) separable into f(i) + g(j), so the 2D
max-plus convolution factors into two 1D sliding passes:

  stage 1:  t[p, r] = max_j rowpad[p, j + r] + w[j]     (slide along columns)
  stage 2:  out[r, c] = max_i tpad[i + c, r] + w[i]     (slide along rows)

with w[k] = -(k - 50)^2 / (4 s) and sentinel (-1e30) padding instead of -inf.

Sharding: output rows are split across the 8 cores (13 rows each, 104 >= 101).
Each core runs both stages restricted to its 13 output rows -- no cross-core
communication. Stage 1 keeps input rows on partitions (101 used): one
broadcast-add (tensor_tensor over a [101, 13, 101] sliding-window AP) plus a
free-dim max-reduce. The [101, 13] result is transposed on the tensor engine,
sentinel-padded to [13, 224], and replicated into a [104, 128] layout
(partition P = cc*13 + r holds tpad[r, cc*13 : cc*13+128]) so stage 2 is
again one broadcast-add + free-dim max-reduce across 104 partitions.

Implementation is raw Bass (no Tile framework): manual semaphores avoid the
Tile entry/exit barrier overhead (~12 us on this toolchain), and all eight
replication gathers increment one shared semaphore so the single-sem-wait
ISA limit is satisfied with standalone wait instructions. The transpose
identity is built on-chip by gpsimd; w arrives pre-replicated from the host.
The replication gathers are spread over all three DMA issuers (SP HWDGE,
ACT HWDGE, and gpsimd SWDGE) so three descriptor generators run in parallel.
"""

import numpy as np

K = 101          # image/kernel size
PAD = 50
S = 13           # output rows per core
NCORES = 8
W = S + K - 1    # 113: window columns each core needs for compute
WT = 128         # transfer width: 512-byte rows
XCOLS = 224      # host-side padded row length (>= 7*13 + 128)
TCOLS = 224      # stage-2 padded t row length (>= 7*13 + 128)
SENT = np.float32(-1.0e30)

_CACHE = {}


def _build_nc():
    import concourse.bass as bass
    import concourse.mybir as mybir

    f32 = mybir.dt.float32
    add = mybir.AluOpType.add
    amax = mybir.AluOpType.max
    nc = bass.Bass(target_bir_lowering=False, debug=False, enable_asserts=False)

    x_in = nc.dram_tensor("x", [K, WT], f32, kind="ExternalInput")
    w_in = nc.dram_tensor("w", [NCORES * S, K], f32, kind="ExternalInput")
    out = nc.dram_tensor("out", [NCORES * S, S], f32, kind="ExternalOutput")

    with (
        nc.sbuf_tensor("xs", [K, WT], f32) as xs,
        nc.sbuf_tensor("wsb", [NCORES * S, K], f32) as wsb,
        nc.sbuf_tensor("ones_k", [K, K], f32) as ones_k,
        nc.sbuf_tensor("idn", [K, K], f32) as idn,
        nc.sbuf_tensor("tmp1", [K, S * K], f32) as tmp1,
        nc.sbuf_tensor("t1", [K, S], f32) as t1,
        nc.sbuf_tensor("tpad", [S, TCOLS], f32) as tpad,
        nc.sbuf_tensor("X", [NCORES * S, WT], f32) as X,
        nc.sbuf_tensor("tmp2", [NCORES * S, S * K], f32) as tmp2,
        nc.sbuf_tensor("osb", [NCORES * S, S], f32) as osb,
        nc.psum_tensor("tp_ps", [S, K], f32) as tp_ps,
        nc.semaphore("s_dx") as s_dx,
        nc.semaphore("s_dw") as s_dw,
        nc.semaphore("s_idn") as s_idn,
        nc.semaphore("s_pe") as s_pe,
        nc.semaphore("s_dve") as s_dve,
        nc.semaphore("s_g") as s_g,
        nc.semaphore("s_out") as s_out,
        nc.Block() as block,
    ):
        xs_win = bass.AP(xs, 0, [[WT, K], [1, S], [1, K]])
        ws_b1 = bass.AP(wsb, 0, [[K, K], [0, S], [1, K]])
        tmp1_w = bass.AP(tmp1, 0, [[S * K, K], [K, S], [1, K]])
        X_win = bass.AP(X, 0, [[WT, NCORES * S], [1, S], [1, K]])
        ws_b2 = bass.AP(wsb, 0, [[K, NCORES * S], [0, S], [1, K]])
        tmp2_w = bass.AP(tmp2, 0, [[S * K, NCORES * S], [K, S], [1, K]])

        def gather(eng, cc):
            return eng.dma_start(
                X[cc * S : (cc + 1) * S, :],
                tpad[0:S, cc * S : cc * S + WT],
            ).then_inc(s_g, 16)

        @block.sync
        def _(sync):
            sync.dma_start(
                bass.AP(xs, 0, [[WT, 51], [1, WT]]),
                bass.AP(x_in, 0, [[WT, 51], [1, WT]]),
            ).then_inc(s_dx, 16)
            sync.dma_start(
                bass.AP(xs, 51 * WT, [[WT, 50], [1, WT]]),
                bass.AP(x_in, 51 * WT, [[WT, 50], [1, WT]]),
            ).then_inc(s_dx, 16)
            sync.wait_ge(s_dve, 2)
            for cc in range(3):
                gather(sync, cc)

        @block.scalar
        def _(scalar):
            scalar.dma_start(wsb[:, :], w_in[:, :]).then_inc(s_dw, 16)
            scalar.wait_ge(s_dve, 2)
            for cc in range(3, 6):
                gather(scalar, cc)
            scalar.wait_ge(s_dve, 3)
            scalar.dma_start(out[:, :], osb[:, :]).then_inc(s_out, 16)

        @block.gpsimd
        def _(gpsimd):
            gpsimd.memset(ones_k[:, :], 1.0)
            gpsimd.drain()
            gpsimd.affine_select(
                idn[:, :],
                ones_k[:, :],
                [[1, K]],
                mybir.AluOpType.is_equal,
                0.0,
                base=0,
                channel_multiplier=-1,
            ).then_inc(s_idn, 1)
            gpsimd.wait_ge(s_dve, 2)
            for cc in range(6, NCORES):
                gather(gpsimd, cc)

        @block.tensor
        def _(tensor):
            tensor.wait_ge(s_idn, 1)
            tensor.wait_ge(s_dve, 1)
            tensor.transpose(tp_ps[:, :], t1[:, :], idn[:, :]).then_inc(s_pe, 1)

        @block.vector
        def _(vector):
            vector.memset(tpad[:, :], float(SENT))
            vector.wait_ge(s_dw, 16)
            vector.wait_ge(s_dx, 32)
            # stage 1: tmp1[p, r, j] = xs[p, r + j] + w[j]
            vector.tensor_tensor(tmp1_w, xs_win, ws_b1, add)
            vector.drain()
            vector.tensor_reduce(
                t1[:, :], tmp1_w, axis=mybir.AxisListType.X, op=amax
            ).then_inc(s_dve, 1)
            vector.wait_ge(s_pe, 1)
            vector.drain()
            # tpad[r, 50 + p] = t1[p, r]
            vector.tensor_copy(tpad[0:S, PAD : PAD + K], tp_ps[:, :]).then_inc(
                s_dve, 1
            )
            vector.wait_ge(s_g, 128)
            # stage 2: tmp2[P, c, i] = X[P, c + i] + w[i]
            vector.tensor_tensor(tmp2_w, X_win, ws_b2, add)
            vector.drain()
            vector.tensor_reduce(
                osb[:, :], tmp2_w, axis=mybir.AxisListType.X, op=amax
            ).then_inc(s_dve, 1)

    return nc


def _prep_in_maps(input, scale):
    inp = np.asarray(input, dtype=np.float32)
    s = np.float32(np.asarray(scale).reshape(()))

    z = (np.arange(K, dtype=np.float32) - np.float32(PAD)).astype(np.float32)
    zsq = (z * z).astype(np.float32)
    wvec = (-zsq / (np.float32(4.0) * s)).astype(np.float32)
    w_rep = np.ascontiguousarray(np.tile(wvec[None, :], (NCORES * S, 1)))

    rowpad = np.full((K, XCOLS), SENT, dtype=np.float32)
    rowpad[:, PAD : PAD + K] = inp

    in_maps = []
    for k in range(NCORES):
        in_maps.append(
            {
                "x": np.ascontiguousarray(rowpad[:, S * k : S * k + WT]),
                "w": w_rep,
            }
        )
    return in_maps


def _unshard(results):
    out_full = np.empty((K, K), dtype=np.float32)
    for k, res in enumerate(results):
        o = np.asarray(res["out"]).reshape(NCORES, S, S)  # [cc, r_loc, c_in]
        block = o.transpose(1, 0, 2).reshape(S, NCORES * S)  # [r_loc, c]
        r0 = S * k
        nrows = min(S, K - r0)
        if nrows <= 0:
            continue
        out_full[r0 : r0 + nrows, :] = block[:nrows, :K]
    return out_full


def kernel(input, scale):
    from concourse.bass_utils import run_bass_kernel_spmd

    if "nc" not in _CACHE:
        _CACHE["nc"] = _build_nc()
    nc = _CACHE["nc"]

    in_maps = _prep_in_maps(input, scale)
    res = run_bass_kernel_spmd(nc, in_maps, core_ids=list(range(NCORES)))
    return _unshard(res.results)


# revision 19
# speedup vs baseline: 1.1632x; 1.0002x over previous
"""Trainium2 Bass kernel for nn_Dilation2D (101x101 grayscale dilation with a
parabolic structuring element).

Math: out[r, c] = max_{i,j} padded[i + c, j + r] + h[i, j] with
h[i, j] = -(z_i^2 + z_j^2) / (4 s) separable into f(i) + g(j), so the 2D
max-plus convolution factors into two 1D sliding passes:

  stage 1:  t[p, r] = max_j rowpad[p, j + r] + w[j]     (slide along columns)
  stage 2:  out[r, c] = max_i tpad[i + c, r] + w[i]     (slide along rows)

with w[k] = -(k - 50)^2 / (4 s) and sentinel (-1e30) padding instead of -inf.

Sharding: output rows are split across the 8 cores (13 rows each, 104 >= 101).
Each core runs both stages restricted to its 13 output rows -- no cross-core
communication. Stage 1 keeps input rows on partitions (101 used): one
broadcast-add (tensor_tensor over a [101, 13, 101] sliding-window AP) plus a
free-dim max-reduce. The [101, 13] result is transposed on the tensor engine,
sentinel-padded to [13, 224], and replicated into a [104, 128] layout
(partition P = cc*13 + r holds tpad[r, cc*13 : cc*13+128]) so stage 2 is
again one broadcast-add + free-dim max-reduce across 104 partitions.

Implementation is raw Bass (no Tile framework): manual semaphores avoid the
Tile entry/exit barrier overhead (~12 us on this toolchain), and all eight
replication gathers increment one shared semaphore so the single-sem-wait
ISA limit is satisfied with standalone wait instructions. The transpose
identity is built on-chip by gpsimd; w arrives pre-replicated from the host.
The replication gathers are spread over all three DMA issuers (SP HWDGE,
ACT HWDGE, and gpsimd SWDGE) so three descriptor generators run in parallel.
"""

import numpy as np

K = 101          # image/kernel size
PAD = 50
S = 13           # output rows per core
NCORES = 8
W = S + K - 1    # 113: window columns each core needs for compute
WT = 128         # transfer width: 512-byte rows
XCOLS = 224      # host-side padded row length (>= 7*13 + 128)
TCOLS = 224      # stage-2 padded t row length (>= 7*13 + 128)
SENT = np.float32(-1.0e30)

_CACHE = {}


def _build_nc():
    import concourse.bass as bass
    import concourse.mybir as mybir

    f32 = mybir.dt.float32
    add = mybir.AluOpType.add
    amax = mybir.AluOpType.max
    nc = bass.Bass(target_bir_lowering=False, debug=False, enable_asserts=False)

    x_in = nc.dram_tensor("x", [K, WT], f32, kind="ExternalInput")
    w_in = nc.dram_tensor("w", [NCORES * S, K], f32, kind="ExternalInput")
    out = nc.dram_tensor("out", [NCORES * S, S], f32, kind="ExternalOutput")

    with (
        nc.sbuf_tensor("xs", [K, WT], f32) as xs,
        nc.sbuf_tensor("wsb", [NCORES * S, K], f32) as wsb,
        nc.sbuf_tensor("ones_k", [K, K], f32) as ones_k,
        nc.sbuf_tensor("idn", [K, K], f32) as idn,
        nc.sbuf_tensor("tmp1", [K, S * K], f32) as tmp1,
        nc.sbuf_tensor("t1", [K, S], f32) as t1,
        nc.sbuf_tensor("tpad", [S, TCOLS], f32) as tpad,
        nc.sbuf_tensor("X", [NCORES * S, WT], f32) as X,
        nc.sbuf_tensor("tmp2", [NCORES * S, S * K], f32) as tmp2,
        nc.sbuf_tensor("osb", [NCORES * S, S], f32) as osb,
        nc.psum_tensor("tp_ps", [S, K], f32) as tp_ps,
        nc.semaphore("s_dx") as s_dx,
        nc.semaphore("s_dw") as s_dw,
        nc.semaphore("s_idn") as s_idn,
        nc.semaphore("s_pe") as s_pe,
        nc.semaphore("s_dve") as s_dve,
        nc.semaphore("s_g") as s_g,
        nc.semaphore("s_g2") as s_g2,
        nc.semaphore("s_out") as s_out,
        nc.Block() as block,
    ):
        xs_win = bass.AP(xs, 0, [[WT, K], [1, S], [1, K]])
        ws_b1 = bass.AP(wsb, 0, [[K, K], [0, S], [1, K]])
        tmp1_w = bass.AP(tmp1, 0, [[S * K, K], [K, S], [1, K]])
        X_win = bass.AP(X, 0, [[WT, NCORES * S], [1, S], [1, K]])
        ws_b2 = bass.AP(wsb, 0, [[K, NCORES * S], [0, S], [1, K]])
        tmp2_w = bass.AP(tmp2, 0, [[S * K, NCORES * S], [K, S], [1, K]])

        def gather(eng, cc, sem):
            return eng.dma_start(
                X[cc * S : (cc + 1) * S, :],
                tpad[0:S, cc * S : cc * S + WT],
            ).then_inc(sem, 16)

        @block.sync
        def _(sync):
            sync.dma_start(
                bass.AP(xs, 0, [[WT, 51], [1, WT]]),
                bass.AP(x_in, 0, [[WT, 51], [1, WT]]),
            ).then_inc(s_dx, 16)
            sync.dma_start(
                bass.AP(xs, 51 * WT, [[WT, 50], [1, WT]]),
                bass.AP(x_in, 51 * WT, [[WT, 50], [1, WT]]),
            ).then_inc(s_dx, 16)
            sync.wait_ge(s_dve, 2)
            for cc in range(3):
                gather(sync, cc, s_g)

        @block.scalar
        def _(scalar):
            scalar.dma_start(wsb[:, :], w_in[:, :]).then_inc(s_dw, 16)
            scalar.wait_ge(s_dve, 2)
            for cc in range(3, 6):
                gather(scalar, cc, s_g)
            scalar.wait_ge(s_dve, 3)
            scalar.dma_start(out[:, :], osb[:, :]).then_inc(s_out, 16)

        @block.gpsimd
        def _(gpsimd):
            gpsimd.memset(ones_k[:, :], 1.0)
            gpsimd.drain()
            gpsimd.affine_select(
                idn[:, :],
                ones_k[:, :],
                [[1, K]],
                mybir.AluOpType.is_equal,
                0.0,
                base=0,
                channel_multiplier=-1,
            ).then_inc(s_idn, 1)
            gpsimd.wait_ge(s_dve, 2)
            for cc in range(6, NCORES):
                gather(gpsimd, cc, s_g2)

        @block.tensor
        def _(tensor):
            tensor.wait_ge(s_idn, 1)
            tensor.wait_ge(s_dve, 1)
            tensor.transpose(tp_ps[:, :], t1[:, :], idn[:, :]).then_inc(s_pe, 1)

        @block.vector
        def _(vector):
            vector.memset(tpad[:, :], float(SENT))
            vector.wait_ge(s_dw, 16)
            vector.wait_ge(s_dx, 32)
            # stage 1: tmp1[p, r, j] = xs[p, r + j] + w[j]
            vector.tensor_tensor(tmp1_w, xs_win, ws_b1, add)
            vector.drain()
            vector.tensor_reduce(
                t1[:, :], tmp1_w, axis=mybir.AxisListType.X, op=amax
            ).then_inc(s_dve, 1)
            vector.wait_ge(s_pe, 1)
            vector.drain()
            # tpad[r, 50 + p] = t1[p, r]
            vector.tensor_copy(tpad[0:S, PAD : PAD + K], tp_ps[:, :]).then_inc(
                s_dve, 1
            )
            vector.wait_ge(s_g, 96)
            vector.wait_ge(s_g2, 32)
            # stage 2: tmp2[P, c, i] = X[P, c + i] + w[i]
            vector.tensor_tensor(tmp2_w, X_win, ws_b2, add)
            vector.drain()
            vector.tensor_reduce(
                osb[:, :], tmp2_w, axis=mybir.AxisListType.X, op=amax
            ).then_inc(s_dve, 1)

    return nc


def _prep_in_maps(input, scale):
    inp = np.asarray(input, dtype=np.float32)
    s = np.float32(np.asarray(scale).reshape(()))

    z = (np.arange(K, dtype=np.float32) - np.float32(PAD)).astype(np.float32)
    zsq = (z * z).astype(np.float32)
    wvec = (-zsq / (np.float32(4.0) * s)).astype(np.float32)
    w_rep = np.ascontiguousarray(np.tile(wvec[None, :], (NCORES * S, 1)))

    rowpad = np.full((K, XCOLS), SENT, dtype=np.float32)
    rowpad[:, PAD : PAD + K] = inp

    in_maps = []
    for k in range(NCORES):
        in_maps.append(
            {
                "x": np.ascontiguousarray(rowpad[:, S * k : S * k + WT]),
                "w": w_rep,
            }
        )
    return in_maps


def _unshard(results):
    out_full = np.empty((K, K), dtype=np.float32)
    for k, res in enumerate(results):
        o = np.asarray(res["out"]).reshape(NCORES, S, S)  # [cc, r_loc, c_in]
        block = o.transpose(1, 0, 2).reshape(S, NCORES * S)  # [r_loc, c]
        r0 = S * k
        nrows = min(S, K - r0)
        if nrows <= 0:
            continue
        out_full[r0 : r0 + nrows, :] = block[:nrows, :K]
    return out_full


def kernel(input, scale):
    from concourse.bass_utils import run_bass_kernel_spmd

    if "nc" not in _CACHE:
        _CACHE["nc"] = _build_nc()
    nc = _CACHE["nc"]

    in_maps = _prep_in_maps(input, scale)
    res = run_bass_kernel_spmd(nc, in_maps, core_ids=list(range(NCORES)))
    return _unshard(res.results)
